# revision 23
# baseline (speedup 1.0000x reference)
"""ParagraphVector negative-sampling loss on 8 Trainium2 NeuronCores.

Strategy (data-parallel over bsz, 128 rows/core):
  - Token embeddings are fetched with SWDGE dma_gather(transpose=True) from a
    bf16 copy of the table, landing as [d=128 partitions, tokens] — directly
    usable as the PE moving operand.
  - int16 gather indices can't span vocab=50000, so the table is split into a
    "lo" half [0, 32767) and "hi" half [32767, 50000), each with a trailing
    all-zero sentinel row used for padding slots.
  - Per batch row r: PE matvec with stationary emb_e[:, r] (M=1) producing the
    row's 1536 token dots.  M=1 matmul outputs can only land on PSUM
    partitions {0,32,64,96}, so rows are processed in groups of 4 and the
    four PSUM rows are compacted into SBUF with strided-partition copies
    (alternating ScalarE/VectorE).
  - loss terms: logsig(+dot) for pos, logsig(-dot) for neg.  Using
    logsig(x) = -softplus(-x), the kernel accumulates softplus(sign*dot) with
    sign=-1 (pos) / +1 (neg) / 0 (pad).  There is no Softplus/Ln ACT table on
    TRN2, so softplus is computed as
        softplus(z) = relu(z) + 2*atanh(w),  w = u/(2+u),  u = exp(-|z|)
    with the atanh odd series in w^2 (w <= 1/3 so it converges fast), using
    the Exp ACT LUT and DVE arithmetic.
  - Pad slots gather the zero sentinel row (dot = 0) with sign 0, contributing
    exactly softplus(0) = ln(2) each; the host subtracts ln(2)*n_pads.
  - Each core emits [sum_softplus, nonzero_pos_count]; the host combines the
    8 partials: loss = (sum - ln2*pads) / ((N_NEG+1)*count).
"""

import math

import ml_dtypes
import numpy as np

import concourse.bass as bass
import concourse.mybir as mybir
import concourse.tile as tile
from concourse import bacc
from concourse.bass_utils import run_bass_kernel_spmd
from concourse.library_config import mlp
from concourse.alu_op_type import AluOpType

N_CORES = 8
BSZ = 1024
D = 128
VOCAB = 50000
NPOS = 256
NNEG = 1280
TOK = NPOS + NNEG
R = BSZ // N_CORES          # rows per core
V_LO = 32767                # lo table covers ids [0, V_LO); hi covers the rest
V_HI = VOCAB - V_LO         # 17233
CH = 8                      # rows per gather chunk

F32 = mybir.dt.float32
BF16 = mybir.dt.bfloat16
I16 = mybir.dt.int16
AF = mybir.ActivationFunctionType

_prog_cache = {}


GCAP = 1024  # max dma_gather idxs per call (SWDGE ring carveout limit)
_DEBUG_DOTS = False


def _bcast4(t_ap, dims):
    """Build a 4-D AP over tile AP `t_ap` with explicit free dims."""
    return bass.AP(t_ap.tensor, t_ap.offset,
                   [list(t_ap.ap[0])] + [[s, n] for s, n in dims])


def _build_program(w_lo, w_hi):
    w = w_lo + w_hi
    nc = bacc.Bacc("TRN2", target_bir_lowering=False)

    t_lo = nc.dram_tensor("t_lo", [V_LO + 1, D], BF16, kind="ExternalInput")
    t_hi = nc.dram_tensor("t_hi", [V_HI + 1, D], BF16, kind="ExternalInput")
    e_flat = nc.dram_tensor("e_flat", [1, R * D], BF16, kind="ExternalInput")
    idx_lo = nc.dram_tensor("idx_lo", [128, R * w_lo // 16], I16, kind="ExternalInput")
    idx_hi = nc.dram_tensor("idx_hi", [128, R * w_hi // 16], I16, kind="ExternalInput")
    sgn = nc.dram_tensor("sign", [128, w], F32, kind="ExternalInput")
    posf = nc.dram_tensor("pos_f32", [128, NPOS], F32, kind="ExternalInput")
    out = nc.dram_tensor("out", [128, 2], F32, kind="ExternalOutput")
    dbg_dots = (
        nc.dram_tensor("dbg_dots", [128, w], F32, kind="ExternalOutput")
        if _DEBUG_DOTS else None
    )
    dbg_acc = (
        nc.dram_tensor("dbg_acc", [128, 2], F32, kind="ExternalOutput")
        if _DEBUG_DOTS else None
    )

    nt_lo = w_lo // 128       # slots per row (lo)
    nt_hi = w_hi // 128
    nt = nt_lo + nt_hi        # dots cols per row
    nk_lo = CH * w_lo // 16   # idx free-dim per chunk
    nk_hi = CH * w_hi // 16
    ncall_lo = CH * w_lo // GCAP
    ncall_hi = CH * w_hi // GCAP
    assert CH * w_lo % GCAP == 0 and CH * w_hi % GCAP == 0

    with tile.TileContext(nc) as tc:
        with (
            tc.tile_pool(name="const", bufs=1) as pc,
            tc.tile_pool(name="io", bufs=2) as pio,
            tc.tile_pool(name="gath", bufs=2) as pg,
            tc.tile_pool(name="work", bufs=1) as pw,
            tc.tile_pool(name="psum", bufs=2, space="PSUM") as pp,
        ):
            sb_sgn = pc.tile([128, w], F32, tag="sgn")
            nc.sync.dma_start(sb_sgn[:], sgn[:])
            sb_posf = pc.tile([128, NPOS], F32, tag="posf")
            nc.sync.dma_start(sb_posf[:], posf[:])
            ones_bf = pc.tile([1, 128], BF16, tag="ones_bf")
            nc.vector.memset(ones_bf[:], 1.0)

            nc.gpsimd.load_library(mlp)

            dots = pw.tile([128, w], F32, tag="dots")

            for k in range(R // CH):
                # stage this chunk's indices and emb_e rows
                ilo = pio.tile([128, nk_lo], I16, tag="ilo")
                nc.sync.dma_start(ilo[:], idx_lo[:, k * nk_lo : (k + 1) * nk_lo])
                ihi = pio.tile([128, nk_hi], I16, tag="ihi")
                nc.sync.dma_start(ihi[:], idx_hi[:, k * nk_hi : (k + 1) * nk_hi])
                ech = pio.tile([1, CH * D], BF16, tag="ech")
                nc.sync.dma_start(ech[:], e_flat[:, k * CH * D : (k + 1) * CH * D])

                # replicate the chunk's emb_e rows across all 128 partitions
                ps_e = pp.tile([128, CH * D], F32, tag="pse")
                for c in range(0, CH * D, 512):
                    nc.tensor.matmul(ps_e[:, c : c + 512], ones_bf[:],
                                     ech[:, c : c + 512])
                erep = pio.tile([128, CH, D], BF16, tag="erep")
                nc.scalar.copy(erep[:, :, :], ps_e[:])

                g_lo = pg.tile([128, CH * nt_lo, D], BF16, tag="glo")
                for c in range(ncall_lo):
                    nc.gpsimd.dma_gather(
                        g_lo[:, c * (GCAP // 128) : (c + 1) * (GCAP // 128), :],
                        t_lo[:], ilo[:, c * (GCAP // 16) : (c + 1) * (GCAP // 16)],
                        GCAP, GCAP, D,
                    )
                g_hi = pg.tile([128, CH * nt_hi, D], BF16, tag="ghi")
                for c in range(ncall_hi):
                    nc.gpsimd.dma_gather(
                        g_hi[:, c * (GCAP // 128) : (c + 1) * (GCAP // 128), :],
                        t_hi[:], ihi[:, c * (GCAP // 16) : (c + 1) * (GCAP // 16)],
                        GCAP, GCAP, D,
                    )

                # dots = sum_d g * e_rep, per gathered slot
                p_lo = pg.tile([128, CH * nt_lo, D], BF16, tag="plo")
                v = nc.vector
                g4 = _bcast4(g_lo[:, :, :], [(nt_lo * D, CH), (D, nt_lo), (1, D)])
                e4 = _bcast4(erep[:, :, :], [(D, CH), (0, nt_lo), (1, D)])
                o4 = _bcast4(p_lo[:, :, :], [(nt_lo * D, CH), (D, nt_lo), (1, D)])
                v.tensor_tensor(o4, g4, e4, AluOpType.mult)
                d4 = bass.AP(dots[:].tensor, dots[:].offset + k * CH * nt,
                             [list(dots[:].ap[0]), [nt, CH], [1, nt_lo]])
                v.tensor_reduce(d4, o4, mybir.AxisListType.X, AluOpType.add)

                p_hi = pg.tile([128, CH * nt_hi, D], BF16, tag="phi")
                g4 = _bcast4(g_hi[:, :, :], [(nt_hi * D, CH), (D, nt_hi), (1, D)])
                e4 = _bcast4(erep[:, :, :], [(D, CH), (0, nt_hi), (1, D)])
                o4 = _bcast4(p_hi[:, :, :], [(nt_hi * D, CH), (D, nt_hi), (1, D)])
                v.tensor_tensor(o4, g4, e4, AluOpType.mult)
                d4 = bass.AP(dots[:].tensor, dots[:].offset + k * CH * nt + nt_lo,
                             [list(dots[:].ap[0]), [nt, CH], [1, nt_hi]])
                v.tensor_reduce(d4, o4, mybir.AxisListType.X, AluOpType.add)

            if dbg_dots is not None:
                nc.sync.dma_start(dbg_dots[:], dots[:])

            # softplus(dots*sign) summed per partition.
            acc = pw.tile([128, 2], F32, tag="acc")
            t1 = pw.tile([128, w], F32, tag="t1")
            t2 = pw.tile([128, w], F32, tag="t2")
            t3 = pw.tile([128, w], F32, tag="t3")
            t4 = pw.tile([128, w], F32, tag="t4")
            t5 = pw.tile([128, w], F32, tag="t5")
            t6 = pw.tile([128, w], F32, tag="t6")

            v = nc.vector
            v.tensor_tensor(t1[:], dots[:], sb_sgn[:], AluOpType.mult)     # z
            v.tensor_scalar_mul(t2[:], t1[:], -1.0)                        # -z
            v.tensor_tensor(t2[:], t1[:], t2[:], AluOpType.max)            # |z|
            nc.scalar.activation(t3[:], t2[:], AF.Exp, scale=-1.0)         # u
            v.tensor_scalar_max(t2[:], t1[:], 0.0)                         # relu(z)
            v.tensor_scalar_add(t4[:], t3[:], 2.0)                         # u+2
            v.reciprocal(t5[:], t4[:])
            v.tensor_tensor(t4[:], t3[:], t5[:], AluOpType.mult)           # w
            v.tensor_tensor(t5[:], t4[:], t4[:], AluOpType.mult)           # w^2
            v.tensor_scalar_mul(t1[:], t5[:], 1.0 / 13.0)
            cur, alt = t1, t6
            for c in (1.0 / 11.0, 1.0 / 9.0, 1.0 / 7.0, 1.0 / 5.0, 1.0 / 3.0):
                v.scalar_tensor_tensor(
                    alt[:], cur[:], c, t5[:], AluOpType.add, AluOpType.mult
                )
                cur, alt = alt, cur
            v.scalar_tensor_tensor(
                alt[:], cur[:], 1.0, t4[:], AluOpType.add, AluOpType.mult
            )                                                              # atanh(w)
            v.scalar_tensor_tensor(
                cur[:], alt[:], 2.0, t2[:], AluOpType.mult, AluOpType.add,
                accum_out=acc[:, 0:1],
            )                                                              # softplus

            # nonzero count of pos tokens per row
            v.tensor_scalar(
                t3[:, 0:NPOS], sb_posf[:], 0.0, 0.0, AluOpType.not_equal,
                AluOpType.add, accum_out=acc[:, 1:2],
            )

            if dbg_acc is not None:
                nc.sync.dma_start(dbg_acc[:], acc[:])

            nc.sync.dma_start(out[:], acc[:])

    nc.compile()
    return nc


def _ceil_mult(x, m):
    return (int(x) + m - 1) // m * m


def _wrap_idx(idx_core, width):
    """[R, width] int16 -> [128, R*width/16] wrapped per 1024-idx gather call."""
    calls = idx_core.reshape(-1, GCAP)                 # [ncalls, 1024] row-major
    wrapped = calls.reshape(-1, GCAP // 16, 16)        # [ncalls, 64, 16]
    flat = wrapped.transpose(2, 0, 1).reshape(16, -1)  # [16, ncalls*64]
    return np.tile(flat, (8, 1)).astype(np.int16)


def kernel(emb_e, emb_table, token_pos, token_neg):
    emb_e = np.asarray(emb_e, dtype=np.float32)
    emb_table = np.asarray(emb_table, dtype=np.float32)
    token_pos = np.asarray(token_pos)
    token_neg = np.asarray(token_neg)

    ids = np.concatenate([token_pos, token_neg], axis=1).astype(np.int32)
    sign_row = np.concatenate(
        [np.full(NPOS, -1.0, np.float32), np.full(NNEG, 1.0, np.float32)]
    )
    signs = np.broadcast_to(sign_row, ids.shape)

    is_lo = ids < V_LO
    n_lo = is_lo.sum(axis=1)
    n_hi = TOK - n_lo
    w_lo = _ceil_mult(n_lo.max(), 128)
    w_hi = _ceil_mult(n_hi.max(), 128)
    w = w_lo + w_hi

    order_lo = np.argsort(~is_lo, axis=1, kind="stable")
    ids_lof = np.take_along_axis(ids, order_lo, axis=1)
    sgn_lof = np.take_along_axis(signs, order_lo, axis=1)
    cols = np.arange(w_lo)[None, :] < n_lo[:, None]
    idx_lo = np.where(cols, ids_lof[:, :w_lo], V_LO).astype(np.int16)
    sgn_lo = np.where(cols, sgn_lof[:, :w_lo], 0.0).astype(np.float32)

    order_hi = np.argsort(is_lo, axis=1, kind="stable")
    ids_hif = np.take_along_axis(ids, order_hi, axis=1) - V_LO
    sgn_hif = np.take_along_axis(signs, order_hi, axis=1)
    cols = np.arange(w_hi)[None, :] < n_hi[:, None]
    idx_hi = np.where(cols, ids_hif[:, :w_hi], V_HI).astype(np.int16)
    sgn_hi = np.where(cols, sgn_hif[:, :w_hi], 0.0).astype(np.float32)

    sign_all = np.concatenate([sgn_lo, sgn_hi], axis=1)
    nt = w // 128

    key = (w_lo, w_hi)
    if key not in _prog_cache:
        _prog_cache[key] = _build_program(w_lo, w_hi)
    nc = _prog_cache[key]

    table_bf16 = emb_table.astype(ml_dtypes.bfloat16)
    zero_row = np.zeros((1, D), ml_dtypes.bfloat16)
    t_lo = np.ascontiguousarray(np.concatenate([table_bf16[:V_LO], zero_row]))
    t_hi = np.ascontiguousarray(np.concatenate([table_bf16[V_LO:], zero_row]))

    in_maps = []
    for c in range(N_CORES):
        rows = slice(c * R, (c + 1) * R)
        # device dots layout: partition = token-within-128-tile,
        # free col = (chunk, row-in-chunk, tile)
        sign_dev = np.ascontiguousarray(
            sign_all[rows].reshape(R // CH, CH, nt, 128)
            .transpose(3, 0, 1, 2).reshape(128, w)
        )
        in_maps.append({
            "t_lo": t_lo,
            "t_hi": t_hi,
            "e_flat": np.ascontiguousarray(emb_e[rows]).astype(
                ml_dtypes.bfloat16).reshape(1, R * D),
            "idx_lo": _wrap_idx(idx_lo[rows], w_lo),
            "idx_hi": _wrap_idx(idx_hi[rows], w_hi),
            "sign": sign_dev,
            "pos_f32": token_pos[rows].astype(np.float32),
        })

    res = run_bass_kernel_spmd(nc, in_maps, list(range(N_CORES)))

    s_dev = sum(float(res.results[c]["out"][:, 0].sum()) for c in range(N_CORES))
    n_nz = sum(float(res.results[c]["out"][:, 1].sum()) for c in range(N_CORES))
    n_pads = BSZ * (w - TOK)
    s_true = s_dev - math.log(2.0) * n_pads
    loss = s_true / (6.0 * n_nz)  # n_token = (N_NEG + 1) * sum(lens), N_NEG = 5
    return np.float32(loss)


if __name__ == "__main__":
    rng = np.random.default_rng(0)
    emb_e = rng.standard_normal((BSZ, D), dtype=np.float32)
    emb_table = rng.standard_normal((VOCAB, D), dtype=np.float32)
    token_pos = rng.integers(0, VOCAB, (BSZ, NPOS), dtype=np.int64)
    token_neg = rng.integers(0, VOCAB, (BSZ, NNEG), dtype=np.int64)
    got = kernel(emb_e=emb_e, emb_table=emb_table,
                 token_pos=token_pos, token_neg=token_neg)

    lens = (token_pos != 0).sum(axis=1)
    n_token = (5 + 1) * lens.sum()
    ep = emb_table[token_pos]
    en = emb_table[token_neg]
    dp = np.einsum("bd,bpd->bp", emb_e, ep)
    dn = -np.einsum("bd,bpd->bp", emb_e, en)

    def logsig(x):
        return -np.logaddexp(0.0, -x)

    s = logsig(dp).sum() + logsig(dn).sum()
    want = -s / n_token
    print("got", got, "want", want, "rel", abs(got - want) / abs(want))


# revision 24
# speedup vs baseline: 1.1276x; 1.1276x over previous
"""ParagraphVector negative-sampling loss on 8 Trainium2 NeuronCores.

Data-parallel over bsz: core c handles rows [128c, 128c+128).

Per core:
  - Token embeddings (1536 per row: 256 pos + 1280 neg) are fetched from a
    bf16 copy of the [50000, 128] table with SWDGE dma_gather.  Gather
    indices are int16; to cover vocab 50000 the gather base points at table
    row 25000 and idx = id - 25000 in [-25000, 25000) — the Q7 descriptor
    address math is a signed MAC, so negative indices address earlier rows
    correctly.  (The descgen trims *trailing* negative indices, so the host
    swaps each 1024-index call's last slot with a same-row same-sign
    non-negative index.)
  - The SWDGE descriptor ring in this runtime holds 1024 descriptors, so
    gathers are issued as 192 calls of 1024 indices (12 calls per 8-row
    chunk; 1536 = 12*128 slots per row exactly).
  - Gathered layout is [token%128 partition, slot, d].  Dots are computed on
    VectorE: multiply by the row's emb_e replicated across partitions (built
    with a K=1 ones outer-product matmul on PE), then a free-axis reduce
    over d.  dots[p, col], col = (chunk, row-in-chunk, slot).
  - loss terms: logsig(+dot) for pos, logsig(-dot) for neg. With
    logsig(x) = -softplus(-x), accumulate softplus(z), z = -sign*dot:
    pos slots (slot 0-1) are negated in place; neg slots keep +dot.
    No Softplus/Ln ACT table exists on TRN2, so softplus is computed as
        softplus(z) = relu(z) + 2*atanh(w),  w = u/(2+u),  u = exp(-|z|)
    using the Exp ACT LUT and a DVE atanh odd series in w^2 (w <= 1/3).
  - Each core outputs acc[128, 2] = per-partition [sum_softplus,
    nonzero-pos-count]; the host sums partials over partitions and cores:
    loss = sum_softplus_total / (6 * count_total).
"""

import ml_dtypes
import numpy as np

import concourse.bass as bass
import concourse.mybir as mybir
import concourse.tile as tile
from concourse import bacc
from concourse.bass_utils import run_bass_kernel_spmd
from concourse.library_config import mlp
from concourse.alu_op_type import AluOpType

N_CORES = 8
BSZ = 1024
D = 128
VOCAB = 50000
NPOS = 256
NNEG = 1280
TOK = NPOS + NNEG           # 1536 = 12 * 128
R = BSZ // N_CORES          # 128 rows per core
V_OFF = 25000               # gather base row; idx = id - V_OFF fits int16
CH = 8                      # rows per chunk
NT = TOK // 128             # 12 slots per row
GCAP = 1024                 # max dma_gather idxs per call (SWDGE ring size)

F32 = mybir.dt.float32
BF16 = mybir.dt.bfloat16
I16 = mybir.dt.int16
AF = mybir.ActivationFunctionType

_DEBUG_DOTS = False
_prog_cache = {}


def _bcast4(t_ap, dims):
    """Build a 4-D AP over tile AP `t_ap` with explicit free dims."""
    return bass.AP(t_ap.tensor, t_ap.offset,
                   [list(t_ap.ap[0])] + [[s, n] for s, n in dims])


def _build_program():
    nc = bacc.Bacc("TRN2", target_bir_lowering=False)

    tbl = nc.dram_tensor("tbl", [VOCAB, D], BF16, kind="ExternalInput")
    e_flat = nc.dram_tensor("e_flat", [1, R * D], BF16, kind="ExternalInput")
    idx = nc.dram_tensor("idx", [128, R * TOK // 16], I16, kind="ExternalInput")
    posf = nc.dram_tensor("pos_f32", [128, NPOS], F32, kind="ExternalInput")
    out = nc.dram_tensor("out", [128, 2], F32, kind="ExternalOutput")
    dbg_dots = (
        nc.dram_tensor("dbg_dots", [128, R * NT], F32, kind="ExternalOutput")
        if _DEBUG_DOTS else None
    )

    nchunk = R // CH              # 16
    nk = CH * TOK // 16           # idx cols per chunk (768)
    ncall = CH * TOK // GCAP      # 12 gather calls per chunk
    w = R * NT                    # dots cols (1536)

    with tile.TileContext(nc) as tc:
        with (
            tc.tile_pool(name="const", bufs=1) as pc,
            tc.tile_pool(name="io", bufs=2) as pio,
            tc.tile_pool(name="gath", bufs=2) as pg,
            tc.tile_pool(name="work", bufs=1) as pw,
            tc.tile_pool(name="psum", bufs=2, space="PSUM") as pp,
        ):
            sb_posf = pc.tile([128, NPOS], F32, tag="posf")
            nc.sync.dma_start(sb_posf[:], posf[:])
            ones_bf = pc.tile([1, 128], BF16, tag="ones_bf")
            nc.vector.memset(ones_bf[:], 1.0)

            nc.gpsimd.load_library(mlp)

            dots = pw.tile([128, w], F32, tag="dots")
            v = nc.vector

            for k in range(nchunk):
                ixc = pio.tile([128, nk], I16, tag="ixc")
                nc.sync.dma_start(ixc[:], idx[:, k * nk : (k + 1) * nk])
                ech = pio.tile([1, CH * D], BF16, tag="ech")
                nc.sync.dma_start(ech[:], e_flat[:, k * CH * D : (k + 1) * CH * D])

                # replicate the chunk's emb_e rows across all 128 partitions
                ps_e = pp.tile([128, CH * D], F32, tag="pse")
                for c in range(0, CH * D, 512):
                    nc.tensor.matmul(ps_e[:, c : c + 512], ones_bf[:],
                                     ech[:, c : c + 512])
                erep = pio.tile([128, CH, D], BF16, tag="erep")
                nc.scalar.copy(erep[:, :, :], ps_e[:])

                g = pg.tile([128, CH * NT, D], BF16, tag="g")
                for c in range(ncall):
                    nc.gpsimd.dma_gather(
                        g[:, c * (GCAP // 128) : (c + 1) * (GCAP // 128), :],
                        tbl[V_OFF:, :], ixc[:, c * (GCAP // 16) : (c + 1) * (GCAP // 16)],
                        GCAP, GCAP, D,
                    )

                # dots[p, k*CH*NT + rc*NT + t] = sum_d g * e_rep
                p_t = pg.tile([128, CH * NT, D], BF16, tag="p")
                g4 = _bcast4(g[:, :, :], [(NT * D, CH), (D, NT), (1, D)])
                e4 = _bcast4(erep[:, :, :], [(D, CH), (0, NT), (1, D)])
                o4 = _bcast4(p_t[:, :, :], [(NT * D, CH), (D, NT), (1, D)])
                v.tensor_tensor(o4, g4, e4, AluOpType.mult)
                d4 = bass.AP(dots[:].tensor, dots[:].offset + k * CH * NT,
                             [list(dots[:].ap[0]), [NT, CH], [1, NT]])
                v.tensor_reduce(d4, o4, mybir.AxisListType.X, AluOpType.add)

            if dbg_dots is not None:
                nc.sync.dma_start(dbg_dots[:], dots[:])

            # z = -sign*dot: negate the pos slots (slot 0-1 of each row)
            zpos = bass.AP(dots[:].tensor, dots[:].offset,
                           [list(dots[:].ap[0]), [NT, R], [1, 2]])
            v.tensor_scalar_mul(zpos, zpos, -1.0)

            # softplus(z) summed per partition
            acc = pw.tile([128, 2], F32, tag="acc")
            t1 = pw.tile([128, w], F32, tag="t1")
            t2 = pw.tile([128, w], F32, tag="t2")
            t3 = pw.tile([128, w], F32, tag="t3")
            t4 = pw.tile([128, w], F32, tag="t4")
            t5 = pw.tile([128, w], F32, tag="t5")
            t6 = pw.tile([128, w], F32, tag="t6")

            z = dots
            v.tensor_scalar_mul(t2[:], z[:], -1.0)                        # -z
            v.tensor_tensor(t2[:], z[:], t2[:], AluOpType.max)            # |z|
            nc.scalar.activation(t3[:], t2[:], AF.Exp, scale=-1.0)        # u
            v.tensor_scalar_max(t2[:], z[:], 0.0)                         # relu(z)
            v.tensor_scalar_add(t4[:], t3[:], 2.0)                        # u+2
            v.reciprocal(t5[:], t4[:])
            v.tensor_tensor(t4[:], t3[:], t5[:], AluOpType.mult)          # w
            v.tensor_tensor(t5[:], t4[:], t4[:], AluOpType.mult)          # w^2
            v.tensor_scalar_mul(t1[:], t5[:], 1.0 / 13.0)
            cur, alt = t1, t6
            for cc in (1.0 / 11.0, 1.0 / 9.0, 1.0 / 7.0, 1.0 / 5.0, 1.0 / 3.0):
                v.scalar_tensor_tensor(
                    alt[:], cur[:], cc, t5[:], AluOpType.add, AluOpType.mult
                )
                cur, alt = alt, cur
            v.scalar_tensor_tensor(
                alt[:], cur[:], 1.0, t4[:], AluOpType.add, AluOpType.mult
            )                                                             # atanh(w)
            v.scalar_tensor_tensor(
                cur[:], alt[:], 2.0, t2[:], AluOpType.mult, AluOpType.add,
                accum_out=acc[:, 0:1],
            )                                                             # softplus

            # nonzero count of pos tokens per partition-row
            v.tensor_scalar(
                t3[:, 0:NPOS], sb_posf[:], 0.0, 0.0, AluOpType.not_equal,
                AluOpType.add, accum_out=acc[:, 1:2],
            )

            nc.sync.dma_start(out[:], acc[:])

    nc.compile()
    return nc


def _wrap_idx(idx_core):
    """[R, TOK] int16 -> [128, R*TOK/16] wrapped per 1024-idx gather call."""
    calls = idx_core.reshape(-1, GCAP)                 # [ncalls, 1024] row-major
    wrapped = calls.reshape(-1, GCAP // 16, 16)        # [ncalls, 64, 16]
    flat = wrapped.transpose(2, 0, 1).reshape(16, -1)  # [16, ncalls*64]
    return np.tile(flat, (8, 1)).astype(np.int16)


def kernel(emb_e, emb_table, token_pos, token_neg):
    emb_e = np.asarray(emb_e, dtype=np.float32)
    emb_table = np.asarray(emb_table, dtype=np.float32)
    token_pos = np.asarray(token_pos)
    token_neg = np.asarray(token_neg)

    ids = np.concatenate([token_pos, token_neg], axis=1).astype(np.int32)
    rel = (ids - V_OFF).astype(np.int16)               # [BSZ, 1536]

    # The descgen trims trailing negative idxs from each call. Ensure the
    # last slot of every 1024-idx call is non-negative by swapping it with a
    # non-negative idx from the same row and same sign class (sum-invariant).
    rel2 = rel.reshape(-1, GCAP)                       # calls, row-major
    ncalls_row = TOK // GCAP if TOK % GCAP == 0 else None
    # call boundaries fall inside rows; compute (row, slot-in-row) of each
    # call's last element to constrain the swap partner.
    flat = rel.reshape(-1)
    n_calls = flat.size // GCAP
    for c in range(n_calls):
        last = c * GCAP + GCAP - 1
        if flat[last] >= 0:
            continue
        row = last // TOK
        pos_in_row = last % TOK
        # same class: pos tokens [0, NPOS), neg tokens [NPOS, TOK)
        lo, hi = (0, NPOS) if pos_in_row < NPOS else (NPOS, TOK)
        seg = flat[row * TOK + lo : row * TOK + hi]
        # candidates also inside call c
        base = row * TOK + lo
        idxs = np.nonzero(seg >= 0)[0]
        idxs = idxs[(base + idxs >= c * GCAP) & (base + idxs < last)]
        assert idxs.size > 0, "no non-negative swap partner in call"
        j = base + idxs[0]
        flat[last], flat[j] = flat[j], flat[last]
    rel = flat.reshape(BSZ, TOK)

    if "prog" not in _prog_cache:
        _prog_cache["prog"] = _build_program()
    nc = _prog_cache["prog"]

    tbl = np.ascontiguousarray(emb_table.astype(ml_dtypes.bfloat16))

    in_maps = []
    for c in range(N_CORES):
        rows = slice(c * R, (c + 1) * R)
        in_maps.append({
            "tbl": tbl,
            "e_flat": np.ascontiguousarray(emb_e[rows]).astype(
                ml_dtypes.bfloat16).reshape(1, R * D),
            "idx": _wrap_idx(rel[rows]),
            "pos_f32": token_pos[rows].astype(np.float32),
        })

    res = run_bass_kernel_spmd(nc, in_maps, list(range(N_CORES)))

    s_dev = sum(float(res.results[c]["out"][:, 0].sum()) for c in range(N_CORES))
    n_nz = sum(float(res.results[c]["out"][:, 1].sum()) for c in range(N_CORES))
    loss = s_dev / (6.0 * n_nz)  # n_token = (N_NEG + 1) * sum(lens), N_NEG = 5
    return np.float32(loss)


if __name__ == "__main__":
    rng = np.random.default_rng(0)
    emb_e = rng.standard_normal((BSZ, D), dtype=np.float32)
    emb_table = rng.standard_normal((VOCAB, D), dtype=np.float32)
    token_pos = rng.integers(0, VOCAB, (BSZ, NPOS), dtype=np.int64)
    token_neg = rng.integers(0, VOCAB, (BSZ, NNEG), dtype=np.int64)
    got = kernel(emb_e=emb_e, emb_table=emb_table,
                 token_pos=token_pos, token_neg=token_neg)

    lens = (token_pos != 0).sum(axis=1)
    n_token = (5 + 1) * lens.sum()
    dp = np.einsum("bd,bpd->bp", emb_e, emb_table[token_pos])
    dn = -np.einsum("bd,bpd->bp", emb_e, emb_table[token_neg])
    s = (-np.logaddexp(0.0, -dp)).sum() + (-np.logaddexp(0.0, -dn)).sum()
    want = -s / n_token
    print("got", got, "want", want, "rel", abs(got - want) / abs(want))
